# revision 1
# baseline (speedup 1.0000x reference)
import sys

import numpy as np

for _p in ("/opt/trn_rl_repo",):
    if _p not in sys.path:
        sys.path.insert(0, _p)

import concourse.bass as bass
import concourse.mybir as mybir
from concourse import bacc
import concourse.tile as tile
from concourse import masks
from concourse.tile_rust import add_dep_helper
from concourse.bass_utils import run_bass_kernel_spmd

B, N, E, H, DH = 64, 197, 768, 12, 64
NPATCH, G14 = 196, 14
NCORES = 8
BPC = B // NCORES  # batches per core
EPS = 1e-6
F32 = mybir.dt.float32
F32R = mybir.dt.float32r
BF16 = mybir.dt.bfloat16
MM_DT = F32R  # dtype for all matmul operands (float32r = full-rate fp32 on PE)

# token partition tiles (all 197 tokens) and patch tiles (tokens 1..196,
# aligned so token tile 0 rows 1..127 == patch tile 0 rows 0..126)
TOK_TILES = ((0, 128), (128, 69))
PAT_TILES = ((0, 127), (127, 69))
GROUPS = BPC // 2  # 2 batches per matmul group -> N=394 rhs (fp32r full rate)
GW = 2 * N  # 394
LINEARIZE = False


def _r32(ap):
    return ap.bitcast(F32R)


def build_nc():
    nc = bacc.Bacc()
    xc = nc.declare_dram_parameter("xc", [BPC, N, E], F32, isOutput=False)
    wqT = nc.declare_dram_parameter("wqT", [E, E], MM_DT, isOutput=False)
    wkT = nc.declare_dram_parameter("wkT", [E, E], MM_DT, isOutput=False)
    wvT = nc.declare_dram_parameter("wvT", [E, E], MM_DT, isOutput=False)
    wva = nc.declare_dram_parameter("wva", [E, 36], MM_DT, isOutput=False)
    d2x = nc.declare_dram_parameter("d2x", [N, G14], F32, isOutput=False)
    d2y = nc.declare_dram_parameter("d2y", [N, G14], F32, isOutput=False)
    bias3 = nc.declare_dram_parameter("bias3", [128, 36], F32, isOutput=False)
    outc = nc.declare_dram_parameter("outc", [BPC, N, E], F32, isOutput=True)

    with tile.TileContext(nc, linearize=LINEARIZE) as tc:
        from contextlib import ExitStack

        with ExitStack() as ctx:
            ep = ctx.enter_context

            wpool = ep(tc.tile_pool(name="w", bufs=1))
            cpool = ep(tc.tile_pool(name="const", bufs=1))
            xnpool = ep(tc.tile_pool(name="xn", bufs=2))
            xnrpool = ep(tc.tile_pool(name="xnr", bufs=4))
            xTpool = ep(tc.tile_pool(name="xT", bufs=1))
            qkpool = ep(tc.tile_pool(name="qk", bufs=1))
            vpool = ep(tc.tile_pool(name="v", bufs=2))
            gpool = ep(tc.tile_pool(name="gxy", bufs=2))
            spool = ep(tc.tile_pool(name="small", bufs=2))
            ppool = ep(tc.tile_pool(name="p", bufs=2))
            pTpool = ep(tc.tile_pool(name="pT", bufs=4))
            tmppool = ep(tc.tile_pool(name="tmp", bufs=2))
            opool = ep(tc.tile_pool(name="out", bufs=2))

            ps_qk = ep(tc.tile_pool(name="ps_qk", bufs=2, space="PSUM"))
            ps_tp = ep(tc.tile_pool(name="ps_tp", bufs=2, space="PSUM"))
            ps_sc = ep(tc.tile_pool(name="ps_sc", bufs=2, space="PSUM"))
            ps_av = ep(tc.tile_pool(name="ps_av", bufs=2, space="PSUM"))

            # ---- constants ----
            ident = cpool.tile([128, 128], F32, tag="ident")
            masks.make_identity(nc, ident[:, :])
            nc.vector.tensor_scalar_add(ident[:, :], ident[:, :], 0.0)
            identb = cpool.tile([128, 128], BF16, tag="identb")
            masks.make_identity(nc, identb[:, :])
            nc.vector.tensor_scalar_add(identb[:, :], identb[:, :], 0.0)

            wq_t, wk_t, wv_t = [], [], []
            for name, dram, lst in (("q", wqT, wq_t), ("k", wkT, wk_t), ("v", wvT, wv_t)):
                for ke in range(6):
                    t = wpool.tile([128, E], MM_DT, tag=f"w{name}{ke}")
                    nc.gpsimd.dma_start(t[:, :], dram[ke * 128 : (ke + 1) * 128, :])
                    # in-place rounding copy: collapses the DMA-queue producers
                    # into the DVE proc so matmul groups stay under the ISA
                    # wait-slot limit
                    nc.vector.tensor_scalar_add(t[:, :], t[:, :], 0.0)
                    lst.append(t)
            wva_t = []
            for ke in range(6):
                t = cpool.tile([128, 36], MM_DT, tag=f"wva{ke}")
                nc.gpsimd.dma_start(t[:, :], wva[ke * 128 : (ke + 1) * 128, :])
                nc.vector.tensor_scalar_add(t[:, :], t[:, :], 0.0)
                wva_t.append(t)
            d2x_t, d2y_t = [], []
            for pt, (poff, pcnt) in enumerate(TOK_TILES):
                tx = cpool.tile([128, G14], F32, tag=f"d2x{pt}")
                ty = cpool.tile([128, G14], F32, tag=f"d2y{pt}")
                nc.gpsimd.dma_start(tx[:pcnt, :], d2x[poff : poff + pcnt, :])
                nc.gpsimd.dma_start(ty[:pcnt, :], d2y[poff : poff + pcnt, :])
                d2x_t.append(tx)
                d2y_t.append(ty)
            bias_t = cpool.tile([128, 36], F32, tag="bias3")
            nc.gpsimd.dma_start(bias_t[:, :], bias3[:, :])

            # ---- main loop over 2-batch groups ----
            for g in range(GROUPS):
                # load x natural, transpose to xT[eb] [128, 394]
                xT = [xTpool.tile([128, GW], MM_DT, tag=f"xT{eb}", name=f"xT{eb}") for eb in range(6)]
                for bi in range(2):
                    b = 2 * g + bi
                    xn = []
                    for tb, (toff, tcnt) in enumerate(TOK_TILES):
                        traw = xnrpool.tile([128, E], F32, tag=f"xnr{tb}", name=f"xnr{tb}")
                        nc.gpsimd.dma_start(traw[:tcnt, :], xc[b, toff : toff + tcnt, :])
                        # stage through DVE so PE transposes wait on DVE only
                        # and the next DMA into this slot waits on DVE only
                        t = xnpool.tile([128, E], F32, tag=f"xn{tb}")
                        nc.vector.tensor_scalar_add(t[:tcnt, :], traw[:tcnt, :], 0.0)
                        xn.append(t)
                    for tb, (toff, tcnt) in enumerate(TOK_TILES):
                        for eb in range(6):
                            tp = ps_tp.tile([128, 128], F32, tag="tp")
                            nc.tensor.transpose(
                                tp[:128, :tcnt],
                                xn[tb][:tcnt, eb * 128 : (eb + 1) * 128],
                                ident[:tcnt, :tcnt],
                            )
                            nc.vector.tensor_scalar_add(xT[eb][:, bi * N + toff : bi * N + toff + tcnt],
                                tp[:128, :tcnt], 0.0)

                # qT / kT projections: [768, 394] as 6 tiles [128, 394]
                qT = [qkpool.tile([128, GW], MM_DT, tag=f"qT{mo}", name=f"qT{mo}") for mo in range(6)]
                qTb = [qkpool.tile([128, GW], BF16, tag=f"qTb{mo}", name=f"qTb{mo}") for mo in range(6)]
                kTb = [qkpool.tile([128, GW], BF16, tag=f"kTb{mo}", name=f"kTb{mo}") for mo in range(6)]
                for wt, is_k in ((wq_t, False), (wk_t, True)):
                    for mo in range(6):
                        ps = ps_qk.tile([128, GW], F32, tag="qk")
                        for ke in range(6):
                            nc.tensor.matmul(
                                ps[:, :],
                                wt[ke][:, mo * 128 : (mo + 1) * 128],
                                xT[ke][:, :],
                                start=(ke == 0),
                                stop=(ke == 5),
                            )
                        if is_k:
                            # fold the 1/sqrt(dh) score scale into k; scores
                            # run in bf16 (f32r needs full-128 contraction)
                            nc.vector.tensor_scalar_mul(kTb[mo][:, :], ps[:, :], 0.125)
                        else:
                            nc.vector.tensor_scalar_add(qT[mo][:, :], ps[:, :], 0.0)
                            nc.vector.tensor_scalar_add(qTb[mo][:, :], ps[:, :], 0.0)

                # v natural layout per batch: tiles [tok, 768]
                v_sb = []
                for bi in range(2):
                    vt = []
                    for tb, (toff, tcnt) in enumerate(TOK_TILES):
                        t = vpool.tile([128, E], BF16, tag=f"v{bi}{tb}")
                        for nb in range(2):
                            ps = ps_qk.tile([128, 384], F32, tag="qk")
                            for ke in range(6):
                                nc.tensor.matmul(
                                    ps[:tcnt, :],
                                    xT[ke][:, bi * N + toff : bi * N + toff + tcnt],
                                    wv_t[ke][:, nb * 384 : (nb + 1) * 384],
                                    start=(ke == 0),
                                    stop=(ke == 5),
                                )
                            nc.vector.tensor_scalar_add(t[:tcnt, nb * 384 : (nb + 1) * 384], ps[:tcnt, :], 0.0)
                        vt.append(t)
                    v_sb.append(vt)

                # per-batch gaussian tables gx (incl alpha), gy  [128, 12*14]
                gx_all, gy_all = [], []
                for bi in range(2):
                    gxt, gyt = [], []
                    for pt, (poff, pcnt) in enumerate(TOK_TILES):
                        ps = ps_tp.tile([128, 36], F32, tag="tp")
                        for ke in range(6):
                            nc.tensor.matmul(
                                ps[:pcnt, :],
                                qT[ke][:, bi * N + poff : bi * N + poff + pcnt],
                                wva_t[ke][:, :],
                                start=(ke == 0),
                                stop=(ke == 5),
                            )
                        # softplus(x) = ln(1 + exp(x)); Softplus has no ACT
                        # func table in this compiler build
                        spa = spool.tile([128, 36], F32, tag="spa")
                        nc.vector.tensor_add(spa[:pcnt, :], ps[:pcnt, :], bias_t[:pcnt, :])
                        spe = spool.tile([128, 36], F32, tag="spe")
                        nc.scalar.activation(
                            spe[:pcnt, :], spa[:pcnt, :],
                            mybir.ActivationFunctionType.Exp,
                        )
                        sp = spool.tile([128, 36], F32, tag="sp")
                        nc.scalar.activation(
                            sp[:pcnt, :], spe[:pcnt, :],
                            mybir.ActivationFunctionType.Ln,
                            bias=1.0,
                        )
                        # rv[p, 2h+c] = 1 / (softplus + 2eps)  (var cols of sp)
                        rv = spool.tile([128, 24], F32, tag="rv")
                        sp3 = sp[:pcnt, :].rearrange("p (h c) -> p h c", c=3)
                        rv3 = rv[:pcnt, :].rearrange("p (h c) -> p h c", c=2)
                        nc.vector.tensor_scalar_add(rv3, sp3[:, :, 0:2], 2.0 * EPS)
                        nc.vector.reciprocal(rv[:pcnt, :], rv[:pcnt, :])
                        # ln(alpha) for folding alpha into gx via exp bias
                        lna = spool.tile([128, 12], F32, tag="lna")
                        nc.scalar.activation(
                            lna[:pcnt, :].unsqueeze(2),
                            sp3[:, :, 2:3],
                            mybir.ActivationFunctionType.Ln,
                        )
                        gx = gpool.tile([128, H * G14], F32, tag=f"gx{bi}{pt}")
                        gy = gpool.tile([128, H * G14], F32, tag=f"gy{bi}{pt}")
                        for h in range(H):
                            nc.scalar.activation(
                                gx[:pcnt, h * G14 : (h + 1) * G14],
                                d2x_t[pt][:pcnt, :],
                                mybir.ActivationFunctionType.Exp,
                                bias=lna[:pcnt, h : h + 1],
                                scale=rv[:pcnt, 2 * h : 2 * h + 1],
                            )
                            nc.scalar.activation(
                                gy[:pcnt, h * G14 : (h + 1) * G14],
                                d2y_t[pt][:pcnt, :],
                                mybir.ActivationFunctionType.Exp,
                                scale=rv[:pcnt, 2 * h + 1 : 2 * h + 2],
                            )
                        if pt == 0:
                            # cls token row must contribute zero bias
                            nc.vector.memset(gx[0:1, :], 0.0)
                        gxt.append(gx)
                        gyt.append(gy)
                    gx_all.append(gxt)
                    gy_all.append(gyt)

                # attention per (batch, head)
                for bi in range(2):
                    out_sb = [
                        opool.tile([128, E], F32, tag=f"o{bi}{it}", name=f"o{bi}{it}") for it in range(2)
                    ]
                    for h in range(H):
                        mo, ro = h // 2, (h % 2) * DH
                        p_sb = [
                            ppool.tile([128, N], BF16, tag=f"p{it}", name=f"p{it}") for it in range(2)
                        ]
                        rs = spool.tile([128, 2], F32, tag="rs")
                        rr = spool.tile([128, 2], F32, tag="rr")
                        for it, (toff, tcnt) in enumerate(TOK_TILES):
                            ps = ps_sc.tile([128, N], F32, tag="sc")
                            nc.tensor.matmul(
                                ps[:tcnt, :],
                                qTb[mo][ro : ro + DH, bi * N + toff : bi * N + toff + tcnt],
                                kTb[mo][ro : ro + DH, bi * N : bi * N + N],
                                start=True,
                                stop=True,
                            )
                            # bias-add doubles as the PSUM->SBUF eviction so the
                            # psum slot is DVE-released (matmul 1-wait limit)
                            gx = gx_all[bi][it]
                            gy = gy_all[bi][it]
                            tmp = tmppool.tile([128, NPATCH], F32, tag="tmp")
                            nc.vector.tensor_mul(
                                tmp[:tcnt, :].rearrange("p (a b) -> p a b", b=G14),
                                gx[:tcnt, h * G14 : (h + 1) * G14]
                                .unsqueeze(2)
                                .broadcast_to([tcnt, G14, G14]),
                                gy[:tcnt, h * G14 : (h + 1) * G14]
                                .unsqueeze(1)
                                .broadcast_to([tcnt, G14, G14]),
                            )
                            s_sb = tmppool.tile([128, N], F32, tag=f"s{it}", name=f"s{it}")
                            nc.vector.tensor_add(
                                s_sb[:tcnt, 1:N], ps[:tcnt, 1:N], tmp[:tcnt, :]
                            )
                            nc.vector.tensor_scalar_add(s_sb[:tcnt, 0:1], ps[:tcnt, 0:1], 0.0)
                            nc.scalar.activation(
                                p_sb[it][:tcnt, :],
                                s_sb[:tcnt, :],
                                mybir.ActivationFunctionType.Exp,
                                accum_out=rs[:tcnt, it : it + 1],
                            )
                            nc.vector.reciprocal(
                                rr[:tcnt, it : it + 1], rs[:tcnt, it : it + 1]
                            )
                        # transpose p -> pT via bf16 DMA transpose (HWDGE)
                        pT = [
                            [
                                pTpool.tile(
                                    [128, 128], BF16,
                                    tag=f"pT{jt}{it}", name=f"pT{jt}{it}",
                                )
                                for it in range(2)
                            ]
                            for jt in range(2)
                        ]
                        for it, (ioff, icnt) in enumerate(TOK_TILES):
                            for jt, (joff, jcnt) in enumerate(TOK_TILES):
                                tpb = ps_tp.tile([128, 128], BF16, tag="tp", name="tpb")
                                nc.tensor.matmul(
                                    tpb[:jcnt, :icnt],
                                    p_sb[it][:icnt, joff : joff + jcnt],
                                    identb[:icnt, :icnt],
                                    is_transpose=True,
                                )
                                nc.vector.tensor_scalar_add(pT[jt][it][:jcnt, 0:icnt], tpb[:jcnt, :icnt], 0.0)
                        # out[i, d] = sum_j p[i,j] v[j,d]; normalize on eviction
                        for it, (ioff, icnt) in enumerate(TOK_TILES):
                            av = ps_av.tile([128, DH], F32, tag="av")
                            for jt, (joff, jcnt) in enumerate(TOK_TILES):
                                nc.tensor.matmul(
                                    av[:icnt, :],
                                    pT[jt][it][:jcnt, 0:icnt],
                                    v_sb[bi][jt][:jcnt, h * DH : (h + 1) * DH],
                                    start=(jt == 0),
                                    stop=(jt == 1),
                                )
                            nc.vector.tensor_scalar_mul(
                                out_sb[it][:icnt, h * DH : (h + 1) * DH],
                                av[:icnt, :],
                                rr[:icnt, it : it + 1],
                            )
                    for it, (toff, tcnt) in enumerate(TOK_TILES):
                        nc.gpsimd.dma_start(
                            outc[2 * g + bi, toff : toff + tcnt, :], out_sb[it][:tcnt, :]
                        )
    nc.compile()
    return nc


_NC_CACHE = None


def _get_nc():
    global _NC_CACHE
    if _NC_CACHE is None:
        _NC_CACHE = build_nc()
    return _NC_CACHE


def _prep_inputs(x, Wq, Wk, Wv, W_var, b_var, W_alpha, b_alpha, diff):
    x = np.asarray(x, np.float32)
    wqT = np.ascontiguousarray(np.asarray(Wq, np.float32).T)
    wkT = np.ascontiguousarray(np.asarray(Wk, np.float32).T)
    wvT = np.ascontiguousarray(np.asarray(Wv, np.float32).T)
    W_var = np.asarray(W_var, np.float32)
    W_alpha = np.asarray(W_alpha, np.float32)
    diff = np.asarray(diff)
    # block-diagonal [768, 36]: col 3h+c = W_var[c] (head h rows), 3h+2 = W_alpha
    wva = np.zeros((E, 36), np.float32)
    for h in range(H):
        sl = slice(h * DH, (h + 1) * DH)
        wva[sl, 3 * h + 0] = W_var[0]
        wva[sl, 3 * h + 1] = W_var[1]
        wva[sl, 3 * h + 2] = W_alpha[0]
    # separable -0.5*d^2 tables from diff (p = px*14+py row-major)
    d2x = np.vstack(
        [np.zeros((1, G14), np.float32), -0.5 * diff[:, ::G14, 0].astype(np.float32)]
    )
    d2y = np.vstack(
        [np.zeros((1, G14), np.float32), -0.5 * diff[:, :G14, 1].astype(np.float32)]
    )
    bias3 = np.tile(
        np.concatenate([np.asarray(b_var, np.float32), np.asarray(b_alpha, np.float32)]),
        (128, H),
    ).astype(np.float32)
    shared = dict(wqT=wqT, wkT=wkT, wvT=wvT, wva=wva, d2x=d2x, d2y=d2y, bias3=bias3)
    in_maps = []
    for c in range(NCORES):
        m = dict(shared)
        m["xc"] = np.ascontiguousarray(x[c * BPC : (c + 1) * BPC])
        in_maps.append(m)
    return in_maps


def run(trace=False, **inputs):
    nc = _get_nc()
    in_maps = _prep_inputs(**inputs)
    res = run_bass_kernel_spmd(nc, in_maps, list(range(NCORES)), trace=trace)
    out = np.concatenate([res.results[c]["outc"] for c in range(NCORES)], axis=0)
    return out, res


def kernel(**inputs):
    out, _ = run(trace=False, **inputs)
    return out



# revision 22
# speedup vs baseline: 1.3680x; 1.3680x over previous
import sys

import numpy as np

for _p in ("/opt/trn_rl_repo",):
    if _p not in sys.path:
        sys.path.insert(0, _p)

import concourse.bass as bass
import concourse.mybir as mybir
from concourse import bacc
import concourse.tile as tile
from concourse import masks
from concourse.bass_utils import run_bass_kernel_spmd

B, N, E, H, DH = 64, 197, 768, 12, 64
NCORES = 8
BPC = B // NCORES  # batches per core
EPS = 1e-6
F32 = mybir.dt.float32
F16 = mybir.dt.float16
BF16 = mybir.dt.bfloat16

# token partition tiles (all 197 tokens incl cls)
TOK = ((0, 128), (128, 69))
GROUPS = BPC // 2  # 2 batches per group
GW = 2 * N  # 394
AF = mybir.ActivationFunctionType


def build_nc():
    nc = bacc.Bacc()
    xc = nc.declare_dram_parameter("xc", [BPC, N, E], F32, isOutput=False)
    wq = nc.declare_dram_parameter("wq", [E, E], BF16, isOutput=False)
    wk = nc.declare_dram_parameter("wk", [E, E], BF16, isOutput=False)
    wv = nc.declare_dram_parameter("wv", [E, E], BF16, isOutput=False)
    wva = nc.declare_dram_parameter("wva", [E, 36], BF16, isOutput=False)
    # l6[h] = L6 block at rows 6h..6h+5, zeros elsewhere (K=72 lhsT variants,
    # sidesteps the PE base-partition-must-be-0/32/64 rule)
    l6 = nc.declare_dram_parameter("l6", [H, 72, N], F16, isOutput=False)
    p2 = nc.declare_dram_parameter("p2", [N, 4], F32, isOutput=False)
    bias3 = nc.declare_dram_parameter("bias3", [128, 36], F32, isOutput=False)
    outc = nc.declare_dram_parameter("outc", [BPC, N, E], F32, isOutput=True)

    with tile.TileContext(nc) as tc:
        from contextlib import ExitStack

        with ExitStack() as ctx:
            ep = ctx.enter_context

            wpool = ep(tc.tile_pool(name="w", bufs=1))
            wrawpool = ep(tc.tile_pool(name="wraw", bufs=2))
            cpool = ep(tc.tile_pool(name="const", bufs=1))
            trawpool = ep(tc.tile_pool(name="traw", bufs=2))
            xTpool = ep(tc.tile_pool(name="xT", bufs=2))
            qkpool = ep(tc.tile_pool(name="qk", bufs=2))
            vpool = ep(tc.tile_pool(name="v", bufs=2))
            spool = ep(tc.tile_pool(name="small", bufs=2))
            rpool = ep(tc.tile_pool(name="r", bufs=4))
            btpool = ep(tc.tile_pool(name="bt", bufs=3))
            epool = ep(tc.tile_pool(name="e", bufs=3))
            opool = ep(tc.tile_pool(name="out", bufs=2))

            # PSUM banks: sc2 (2-bank slots) 2x2 + arg 2 + av 2 = 8
            ps_big = ep(tc.tile_pool(name="ps_big", bufs=2, space="PSUM"))
            ps_arg = ep(tc.tile_pool(name="ps_arg", bufs=2, space="PSUM"))
            ps_av = ep(tc.tile_pool(name="ps_av", bufs=2, space="PSUM"))

            # ---- constants ----
            identf = cpool.tile([128, 128], F32, tag="identf")
            masks.make_identity(nc, identf[:, :])
            nc.vector.tensor_scalar_add(identf[:, :], identf[:, :], 0.0)
            identb = cpool.tile([128, 128], BF16, tag="identb")
            masks.make_identity(nc, identb[:, :])
            nc.vector.tensor_scalar_add(identb[:, :], identb[:, :], 0.0)
            identh = cpool.tile([128, 128], F16, tag="identh")
            masks.make_identity(nc, identh[:, :])
            nc.vector.tensor_scalar_add(identh[:, :], identh[:, :], 0.0)

            wq_t, wk_t, wv_t = [], [], []
            for name, dram, lst in (("q", wq, wq_t), ("k", wk, wk_t), ("v", wv, wv_t)):
                for ke in range(6):
                    traw = wrawpool.tile([128, E], BF16, tag="wraw", name="wraw")
                    nc.gpsimd.dma_start(traw[:, :], dram[ke * 128 : (ke + 1) * 128, :])
                    # stage through DVE so matmuls wait on DVE, not DMA queues
                    t = wpool.tile([128, E], BF16, tag=f"w{name}{ke}", name=f"w{name}{ke}")
                    nc.vector.tensor_scalar_add(t[:, :], traw[:, :], 0.0)
                    lst.append(t)
            wva_t = []
            for ke in range(6):
                traw = cpool.tile([128, 36], BF16, tag=f"wvar{ke}", name=f"wvar{ke}")
                nc.gpsimd.dma_start(traw[:, :], wva[ke * 128 : (ke + 1) * 128, :])
                t = cpool.tile([128, 36], BF16, tag=f"wva{ke}", name=f"wva{ke}")
                nc.vector.tensor_scalar_add(t[:, :], traw[:, :], 0.0)
                wva_t.append(t)
            l6_t = []
            for h in range(H):
                l6r = cpool.tile([72, N], F16, tag=f"l6r{h}", name=f"l6r{h}")
                nc.gpsimd.dma_start(l6r[:, :], l6[h, :, :])
                t = cpool.tile([72, N], F16, tag=f"l6t{h}", name=f"l6t{h}")
                nc.vector.tensor_scalar_add(t[:, :], l6r[:, :], 0.0)
                l6_t.append(t)
            p2_t = []
            for tt, (toff, tcnt) in enumerate(TOK):
                t = cpool.tile([128, 4], F32, tag=f"p2{tt}")
                nc.gpsimd.dma_start(t[:tcnt, :], p2[toff : toff + tcnt, :])
                p2_t.append(t)
            bias_t = cpool.tile([128, 36], F32, tag="bias3")
            nc.gpsimd.dma_start(bias_t[:, :], bias3[:, :])

            # ---- main loop over 2-batch groups ----
            for g in range(GROUPS):
                # --- load x and transpose to xT[eb] [128, 394] bf16 ---
                traw = [[None, None], [None, None]]
                for bi in range(2):
                    for tb, (toff, tcnt) in enumerate(TOK):
                        t = trawpool.tile([128, E], F32, tag=f"tr{bi}{tb}", name=f"tr{bi}{tb}")
                        nc.gpsimd.dma_start(t[:tcnt, :], xc[2 * g + bi, toff : toff + tcnt, :])
                        traw[bi][tb] = t
                xT = []
                for eb in range(6):
                    tp = ps_big.tile([128, GW], F32, tag="big", name="tpx")
                    first, last = (0, 0), (1, 1)
                    for bi in range(2):
                        for tb, (toff, tcnt) in enumerate(TOK):
                            nc.tensor.matmul(
                                tp[:128, bi * N + toff : bi * N + toff + tcnt],
                                traw[bi][tb][:tcnt, eb * 128 : (eb + 1) * 128],
                                identf[:tcnt, :tcnt],
                                is_transpose=True,
                                start=((bi, tb) == first),
                                stop=((bi, tb) == last),
                            )
                    t = xTpool.tile([128, GW], BF16, tag=f"xT{eb}", name=f"xT{eb}")
                    nc.vector.tensor_scalar_add(t[:, :], tp[:, :], 0.0)
                    xT.append(t)

                # --- q/k projections -> qTb/kTb [128, 394] bf16 (k prescaled 1/8) ---
                qTb, kTb = [], []
                for wt, lst, nm in ((wq_t, qTb, "q"), (wk_t, kTb, "k")):
                    for mo in range(6):
                        ps = ps_big.tile([128, GW], F32, tag="big", name="psqk")
                        for ke in range(6):
                            nc.tensor.matmul(
                                ps[:, :],
                                wt[ke][:, mo * 128 : (mo + 1) * 128],
                                xT[ke][:, :],
                                start=(ke == 0),
                                stop=(ke == 5),
                            )
                        t = qkpool.tile([128, GW], BF16, tag=f"{nm}T{mo}", name=f"{nm}T{mo}")
                        nc.vector.tensor_scalar_add(t[:, :], ps[:, :], 0.0)
                        lst.append(t)

                # --- v natural layout with interleaved ones col: [tok, 12*65] bf16 ---
                v_sb = [[None, None], [None, None]]
                for bi in range(2):
                    for tb, (toff, tcnt) in enumerate(TOK):
                        t = vpool.tile([128, H * 65], BF16, tag=f"v{bi}{tb}", name=f"v{bi}{tb}")
                        tv = t[:tcnt, :].rearrange("p (h c) -> p h c", c=65)
                        for nb in range(2):
                            ps = ps_arg.tile([128, 384], F32, tag="arg", name="psv")
                            for ke in range(6):
                                nc.tensor.matmul(
                                    ps[:tcnt, :],
                                    xT[ke][:, bi * N + toff : bi * N + toff + tcnt],
                                    wv_t[ke][:, nb * 384 : (nb + 1) * 384],
                                    start=(ke == 0),
                                    stop=(ke == 5),
                                )
                            nc.vector.tensor_scalar_add(
                                tv[:, nb * 6 : (nb + 1) * 6, 0:64],
                                ps[:tcnt, :].rearrange("p (h c) -> p h c", c=64),
                                0.0,
                            )
                        nc.gpsimd.memset(tv[:, :, 64:65], 1.0)
                        v_sb[bi][tb] = t

                # --- gaussian params -> R_T[bi] [72, 197] f16 (rows 6h+k) ---
                R_T = []
                for bi in range(2):
                    rtps = ps_arg.tile([72, N], F16, tag="arg", name="rtps")
                    for pt, (poff, pcnt) in enumerate(TOK):
                        p36 = ps_arg.tile([128, 36], F32, tag="arg", name="p36")
                        for ke in range(6):
                            nc.tensor.matmul(
                                p36[:pcnt, :],
                                qTb[ke][:, bi * N + poff : bi * N + poff + pcnt],
                                wva_t[ke][:, :],
                                start=(ke == 0),
                                stop=(ke == 5),
                            )
                        # softplus(x) = ln(1 + exp(x))
                        spa = spool.tile([128, 36], F32, tag="spa")
                        nc.vector.tensor_add(spa[:pcnt, :], p36[:pcnt, :], bias_t[:pcnt, :])
                        spe = spool.tile([128, 36], F32, tag="spe")
                        nc.scalar.activation(spe[:pcnt, :], spa[:pcnt, :], AF.Exp)
                        sp = spool.tile([128, 36], F32, tag="sp")
                        nc.scalar.activation(sp[:pcnt, :], spe[:pcnt, :], AF.Ln, bias=1.0)
                        sp3 = sp[:pcnt, :].rearrange("p (h c) -> p h c", c=3)
                        # rv[p, 2h+c] = 1/(softplus + 2eps)
                        rv = spool.tile([128, 24], F32, tag="rv")
                        rv3 = rv[:pcnt, :].rearrange("p (h c) -> p h c", c=2)
                        nc.vector.tensor_scalar_add(rv3, sp3[:, :, 0:2], 2.0 * EPS)
                        nc.vector.reciprocal(rv[:pcnt, :], rv[:pcnt, :])
                        rvx = rv3[:, :, 0:1]
                        rvy = rv3[:, :, 1:2]
                        # ln(alpha)
                        lna = spool.tile([128, 12], F32, tag="lna")
                        nc.scalar.activation(lna[:pcnt, :].unsqueeze(2), sp3[:, :, 2:3], AF.Ln)
                        # R rows per head: [lna-0.5(rvx*px^2+rvy*py^2), rvx*px, -0.5rvx,
                        #                   rvy*py, -0.5rvy, -40]
                        px = p2_t[pt][:pcnt, 0:1]
                        px2 = p2_t[pt][:pcnt, 1:2]
                        py = p2_t[pt][:pcnt, 2:3]
                        py2 = p2_t[pt][:pcnt, 3:4]
                        rpre = rpool.tile([128, 72], F16, tag="rpre")
                        r6 = rpre[:pcnt, :].rearrange("p (h k) -> p h k", k=6)
                        nc.gpsimd.tensor_scalar_mul(r6[:, :, 1:2], rvx, px)
                        nc.gpsimd.tensor_scalar_mul(r6[:, :, 3:4], rvy, py)
                        nc.gpsimd.tensor_scalar_mul(r6[:, :, 2:3], rvx, -0.5)
                        nc.gpsimd.tensor_scalar_mul(r6[:, :, 4:5], rvy, -0.5)
                        ta = spool.tile([128, 12], F32, tag="ta")
                        tb2 = spool.tile([128, 12], F32, tag="tb2")
                        nc.gpsimd.tensor_scalar_mul(ta[:pcnt, :].unsqueeze(2), rvx, px2)
                        nc.gpsimd.tensor_scalar_mul(tb2[:pcnt, :].unsqueeze(2), rvy, py2)
                        tc2 = spool.tile([128, 12], F32, tag="tc2")
                        nc.gpsimd.tensor_add(tc2[:pcnt, :], ta[:pcnt, :], tb2[:pcnt, :])
                        nc.gpsimd.tensor_scalar_mul(tc2[:pcnt, :], tc2[:pcnt, :], -0.5)
                        nc.gpsimd.tensor_add(
                            r6[:, :, 0:1], tc2[:pcnt, :].unsqueeze(2), lna[:pcnt, :].unsqueeze(2)
                        )
                        nc.gpsimd.memset(r6[:, :, 5:6], -40.0)
                        if pt == 0:
                            # cls query col: zero linear terms, force R0 (and keep
                            # R5) at -40 so bias underflows to 0 for i=0 and (0,0)
                            r60 = rpre[0:1, :].rearrange("p (h k) -> p h k", k=6)
                            nc.gpsimd.memset(r60[:, :, 0:5], 0.0)
                            nc.gpsimd.memset(r60[:, :, 0:1], -40.0)
                        nc.tensor.matmul(
                            rtps[:72, poff : poff + pcnt],
                            rpre[:pcnt, :72],
                            identh[:pcnt, :pcnt],
                            is_transpose=True,
                            start=(pt == 0),
                            stop=(pt == 1),
                        )
                    t = rpool.tile([72, N], F16, tag="rT", name="rT")
                    nc.vector.tensor_scalar_add(t[:, :], rtps[:, :], 0.0)
                    R_T.append(t)

                # --- attention ---
                for bi in range(2):
                    out_sb = [
                        opool.tile([128, E], F32, tag=f"o{bi}{it}", name=f"o{bi}{it}")
                        for it in range(2)
                    ]
                    for hg in range(2):  # head groups of 6
                        av = [
                            ps_av.tile([128, 6 * 65], F32, tag="av", name=f"av{it}")
                            for it in range(2)
                        ]
                        for pp in range(3):  # head pairs within the group
                            h0 = 6 * hg + 2 * pp
                            mo = h0 // 2
                            e_t = []
                            for jt, (joff, jcnt) in enumerate(TOK):
                                # heads of a pair use lhsT bases 0/64 -> separate
                                # banks (same-bank base switch faults the PE)
                                ps = ps_big.tile([128, 1024], F32, tag="big", name="pssc")
                                pa = ps_arg.tile([128, GW], F32, tag="arg", name="psarg")
                                for hh in range(2):
                                    ro = 64 * hh
                                    h = h0 + hh
                                    nc.tensor.matmul(
                                        ps[:jcnt, 512 * hh : 512 * hh + N],
                                        kTb[mo][ro : ro + 64, bi * N + joff : bi * N + joff + jcnt],
                                        qTb[mo][ro : ro + 64, bi * N : bi * N + N],
                                        start=True,
                                        stop=False,
                                    )
                                    nc.tensor.matmul(
                                        pa[:jcnt, hh * N : (hh + 1) * N],
                                        l6_t[h][:, joff : joff + jcnt],
                                        R_T[bi][:, :],
                                        start=(hh == 0),
                                        stop=(hh == 1),
                                    )
                                bt = btpool.tile([128, GW], BF16, tag="bt", name="bt")
                                nc.scalar.activation(bt[:jcnt, :], pa[:jcnt, :], AF.Exp)
                                for hh in range(2):
                                    nc.tensor.matmul(
                                        ps[:jcnt, 512 * hh : 512 * hh + N],
                                        identb[:jcnt, :jcnt],
                                        bt[:jcnt, hh * N : (hh + 1) * N],
                                        start=False,
                                        stop=True,
                                    )
                                e = epool.tile([128, GW], BF16, tag=f"e{jt}", name=f"e{jt}")
                                nc.scalar.activation(
                                    e[:jcnt, :].rearrange("p (two n) -> p two n", n=N),
                                    ps[:jcnt, :].rearrange("p (two n) -> p two n", n=512)[
                                        :, :, 0:N
                                    ],
                                    AF.Exp,
                                )
                                e_t.append(e)
                            for it, (ioff, icnt) in enumerate(TOK):
                                for hh in range(2):
                                    h = h0 + hh
                                    col = (2 * pp + hh) * 65
                                    for jt, (joff, jcnt) in enumerate(TOK):
                                        nc.tensor.matmul(
                                            av[it][:icnt, col : col + 65],
                                            e_t[jt][:jcnt, hh * N + ioff : hh * N + ioff + icnt],
                                            v_sb[bi][jt][:jcnt, h * 65 : h * 65 + 65],
                                            start=(pp == 0 and hh == 0 and jt == 0),
                                            stop=(pp == 2 and hh == 1 and jt == 1),
                                        )
                        # normalize 6 heads at once per token tile
                        for it, (ioff, icnt) in enumerate(TOK):
                            av3 = av[it][:icnt, :].rearrange("p (h c) -> p h c", c=65)
                            rr = spool.tile([128, 6], F32, tag="rr")
                            nc.vector.reciprocal(rr[:icnt, :].unsqueeze(2), av3[:, :, 64:65])
                            nc.vector.tensor_mul(
                                out_sb[it][:icnt, hg * 384 : (hg + 1) * 384].rearrange(
                                    "p (h d) -> p h d", d=64
                                ),
                                av3[:, :, 0:64],
                                rr[:icnt, :].unsqueeze(2).broadcast_to([icnt, 6, 64]),
                            )
                    for it, (toff, tcnt) in enumerate(TOK):
                        nc.gpsimd.dma_start(
                            outc[2 * g + bi, toff : toff + tcnt, :], out_sb[it][:tcnt, :]
                        )
    nc.compile()
    return nc


_NC_CACHE = None


def _get_nc():
    global _NC_CACHE
    if _NC_CACHE is None:
        _NC_CACHE = build_nc()
    return _NC_CACHE


def _prep_inputs(x, Wq, Wk, Wv, W_var, b_var, W_alpha, b_alpha, diff):
    import ml_dtypes

    bf16 = ml_dtypes.bfloat16
    x = np.asarray(x, np.float32)
    wq = np.ascontiguousarray(np.asarray(Wq, np.float32).T).astype(bf16)
    wk = np.ascontiguousarray(np.asarray(Wk, np.float32).T * 0.125).astype(bf16)
    wv = np.ascontiguousarray(np.asarray(Wv, np.float32).T).astype(bf16)
    W_var = np.asarray(W_var, np.float32)
    W_alpha = np.asarray(W_alpha, np.float32)
    diff = np.asarray(diff)
    # block-diagonal [768, 36]: cols 3h+{0,1,2} = W_var[0], W_var[1], W_alpha
    wva = np.zeros((E, 36), np.float32)
    for h in range(H):
        sl = slice(h * DH, (h + 1) * DH)
        wva[sl, 3 * h + 0] = W_var[0]
        wva[sl, 3 * h + 1] = W_var[1]
        wva[sl, 3 * h + 2] = W_alpha[0]
    wva = wva.astype(bf16)
    # grid coordinates per token (derived from diff against patch 0 at (0,0))
    pxp = np.sqrt(diff[:, 0, 0].astype(np.float64)).astype(np.float32)  # (196,)
    pyp = np.sqrt(diff[:, 0, 1].astype(np.float64)).astype(np.float32)
    px = np.concatenate([[0.0], pxp]).astype(np.float32)  # (197,) token-indexed
    py = np.concatenate([[0.0], pyp]).astype(np.float32)
    # L6 [6, 197]: col j>=1 -> [1, px, px^2, py, py^2, 0]; col 0 (cls) -> e_5
    l6a = np.zeros((6, N), np.float32)
    l6a[0, 1:] = 1.0
    l6a[1, 1:] = px[1:]
    l6a[2, 1:] = px[1:] ** 2
    l6a[3, 1:] = py[1:]
    l6a[4, 1:] = py[1:] ** 2
    l6a[5, 0] = 1.0
    # 12 block lhsT variants: l6[h] has L6 at rows 6h..6h+5, zeros elsewhere
    l6 = np.zeros((H, 72, N), np.float32)
    for h in range(H):
        l6[h, 6 * h : 6 * h + 6] = l6a
    l6 = l6.astype(np.float16)
    p2 = np.stack([px, px**2, py, py**2], axis=1).astype(np.float32)  # (197, 4)
    bias3 = np.tile(
        np.concatenate([np.asarray(b_var, np.float32), np.asarray(b_alpha, np.float32)]),
        (128, H),
    ).astype(np.float32)
    shared = dict(wq=wq, wk=wk, wv=wv, wva=wva, l6=l6, p2=p2, bias3=bias3)
    in_maps = []
    for c in range(NCORES):
        m = dict(shared)
        m["xc"] = np.ascontiguousarray(x[c * BPC : (c + 1) * BPC])
        in_maps.append(m)
    return in_maps


def run(trace=False, **inputs):
    nc = _get_nc()
    in_maps = _prep_inputs(**inputs)
    res = run_bass_kernel_spmd(nc, in_maps, list(range(NCORES)), trace=trace)
    out = np.concatenate([res.results[c]["outc"] for c in range(NCORES)], axis=0)
    return out, res


def kernel(**inputs):
    out, _ = run(trace=False, **inputs)
    return out
